# revision 1
# baseline (speedup 1.0000x reference)
"""AttentionTCCNet Trainium2 Bass kernel.

Key algebraic fact exploited: the per-step attention adds a *scalar*
(att_h) to every softmax logit, so the softmax weights -- and hence the
attended frame x_t -- are constant across the 16 recurrence steps.  The
computation therefore reduces to a ConvLSTM recurrence whose per-step cost
is a 128->512ch 5x5 conv over the hidden state (13.4 GFLOP/step), plus a
one-time x-path conv and a small CNN tail.

Device kernel: the 16-step ConvLSTM recurrence (conv as 4og x 25offset
stationary-weight matmuls in bf16, fp32 PSUM accumulation, pointwise LSTM
math on Scalar/Vector engines), producing mean-over-time hidden state.
Host: input attention prep (elementwise/stats), the tiny x-path conv, and
the CNN tail (maxpool + 2 convs + normalize), all exact fp32.

SPMD over 8 NeuronCores (replicated recurrence; output read from core 0).
"""

import numpy as np
import ml_dtypes

import concourse.bass as bass
import concourse.mybir as mybir
import concourse.tile as tile
from concourse.bass_utils import run_bass_kernel_spmd

# ---------------------------------------------------------------------------
# Workaround for this container's walrus accepting only ONE SyncWait per
# instruction: split any multi-wait instruction emitted by Tile's semaphore
# assigner into single-wait NoOp carriers inserted immediately before it.
# ---------------------------------------------------------------------------
from concourse.tile import ScopedClock

_MAX_WAITS = 1
_wsplit_counter = [0]


def _split_waits_in_list(insts):
    new = []
    for inst in insts:
        si = getattr(inst, "sync_info", None)
        if si is not None and si.on_wait and len(si.on_wait) > _MAX_WAITS:
            waits = list(si.on_wait)
            for w in waits[:-_MAX_WAITS]:
                _wsplit_counter[0] += 1
                new.append(
                    mybir.InstNoOp(
                        name=f"I-wsplit-{_wsplit_counter[0]}",
                        engine=inst.engine,
                        sync_info=mybir.SyncInfo(on_wait=[w], on_update=[]),
                    )
                )
            si.on_wait = waits[-_MAX_WAITS:]
        new.append(inst)
    insts[:] = new


_orig_lower = tile.TileContext._lower_ordered_insts


def _patched_lower(self, ordered):
    for insts in ordered.values():
        _split_waits_in_list(insts)
    return _orig_lower(self, ordered)


def _patched_drain_and_barrier(self, tick_clock, wait_clock):
    nc = self.nc
    drain_inst = nc.sync.drain()
    wait_clock.add_sem_waits(
        drain_inst.ins, ScopedClock({None: tick_clock.global_clock})
    )
    si = drain_inst.ins.sync_info
    if si is not None and si.on_wait and len(si.on_wait) > _MAX_WAITS:
        waits = list(si.on_wait)
        si.on_wait = waits[:_MAX_WAITS]
        for w in waits[_MAX_WAITS:]:
            extra = nc.sync.drain()
            extra.ins.sync_info = mybir.SyncInfo(on_wait=[w], on_update=[])
    nc.all_engine_barrier()
    assert self.sems is not None
    popped = nc._tile_sem_poison_stack.pop()
    assert popped is self._sem_poison
    nc.clear_and_free_semaphores(list(self.sems.allocated().values()))
    nc.all_engine_barrier()


if tile.TileContext._lower_ordered_insts is not _patched_lower:
    tile.TileContext._lower_ordered_insts = _patched_lower
    tile.TileContext._drain_and_barrier = _patched_drain_and_barrier

# ---------------------------------------------------------------------------

N_CORES = 8
T, HS, H, W = 16, 128, 64, 64
HW = H * W  # 4096
N_CHUNK = 8          # spatial chunks of 8 rows x 64 cols = 512 free
CH_FREE = 512
PADW = 68            # 64 + 2*2 padded layout

FP32 = mybir.dt.float32
BF16 = mybir.dt.bfloat16

_nc_cache = [None]


def build_nc():
    if _nc_cache[0] is not None:
        return _nc_cache[0]
    nc = bass.Bass(num_devices=N_CORES)
    wh_d = nc.dram_tensor("wh", [128, 4 * 25 * 128], BF16, kind="ExternalInput")
    gx_d = nc.dram_tensor("gx", [4, 128, HW], FP32, kind="ExternalInput")
    out_d = nc.dram_tensor("hmean", [128, HW], FP32, kind="ExternalOutput")

    with tile.TileContext(nc) as tc:
        with (
            tc.tile_pool(name="const", bufs=1) as cpool,
            tc.tile_pool(name="hbuf", bufs=2) as hpool,
            tc.tile_pool(name="tmp", bufs=2) as tpool,
            tc.tile_pool(name="psum", bufs=2, space="PSUM") as ppool,
        ):
            wh = cpool.tile([128, 4 * 25 * 128], BF16)
            gx = cpool.tile([128, 4, HW], FP32)
            c_st = cpool.tile([128, HW], FP32)
            hsum = cpool.tile([128, HW], FP32)
            nc.sync.dma_start(wh[:], wh_d[:])
            nc.sync.dma_start(gx[:], gx_d.ap().rearrange("a p h -> p a h"))

            h_pad = None
            for t in range(T):
                if t < T - 1:
                    h_new = hpool.tile([128, PADW, PADW], BF16, tag="hpad")
                    nc.gpsimd.memset(h_new[:], 0.0)
                else:
                    h_new = None

                for ch in range(N_CHUNK):
                    r0 = ch * 8
                    cs = ch * CH_FREE
                    acts = []  # sigmoid(i), sigmoid(f), sigmoid(o), tanh(g)
                    if t == 0:
                        # h == 0: gates are exactly gx
                        for og in range(4):
                            fn = (
                                mybir.ActivationFunctionType.Tanh
                                if og == 3
                                else mybir.ActivationFunctionType.Sigmoid
                            )
                            a = tpool.tile([128, CH_FREE], FP32, tag=f"act{og}")
                            nc.scalar.activation(
                                a[:], gx[:, og, cs : cs + CH_FREE], fn
                            )
                            acts.append(a)
                    else:
                        for og in range(4):
                            ps = ppool.tile([128, CH_FREE], FP32, tag=f"ps{og}")
                            for off in range(25):
                                ky, kx = off // 5, off % 5
                                base = (og * 25 + off) * 128
                                nc.tensor.matmul(
                                    ps[:],
                                    wh[:, base : base + 128],
                                    h_pad[:, r0 + ky : r0 + ky + 8, kx : kx + 64],
                                    start=(off == 0),
                                    stop=(off == 24),
                                )
                            g_sb = tpool.tile([128, CH_FREE], FP32, tag=f"gs{og}")
                            nc.vector.tensor_add(
                                g_sb[:], ps[:], gx[:, og, cs : cs + CH_FREE]
                            )
                            fn = (
                                mybir.ActivationFunctionType.Tanh
                                if og == 3
                                else mybir.ActivationFunctionType.Sigmoid
                            )
                            a = tpool.tile([128, CH_FREE], FP32, tag=f"act{og}")
                            nc.scalar.activation(a[:], g_sb[:], fn)
                            acts.append(a)

                    i_s, f_s, o_s, g_t = acts
                    c_sl = c_st[:, cs : cs + CH_FREE]
                    m2 = tpool.tile([128, CH_FREE], FP32, tag="m2")
                    nc.vector.tensor_mul(m2[:], i_s[:], g_t[:])
                    if t == 0:
                        nc.vector.tensor_copy(c_sl, m2[:])
                    else:
                        m1 = tpool.tile([128, CH_FREE], FP32, tag="m1")
                        nc.vector.tensor_mul(m1[:], f_s[:], c_sl)
                        nc.vector.tensor_add(c_sl, m1[:], m2[:])
                    tc_t = tpool.tile([128, CH_FREE], FP32, tag="tc")
                    nc.scalar.activation(
                        tc_t[:], c_sl, mybir.ActivationFunctionType.Tanh
                    )
                    hf = tpool.tile([128, CH_FREE], FP32, tag="hf")
                    nc.vector.tensor_mul(hf[:], o_s[:], tc_t[:])
                    hs_sl = hsum[:, cs : cs + CH_FREE]
                    if t == 0:
                        nc.vector.tensor_copy(hs_sl, hf[:])
                    else:
                        nc.vector.tensor_add(hs_sl, hs_sl, hf[:])
                    if h_new is not None:
                        nc.vector.tensor_copy(
                            h_new[:, 2 + r0 : 2 + r0 + 8, 2:66],
                            hf[:].rearrange("p (r c) -> p r c", r=8),
                        )
                h_pad = h_new

            nc.scalar.mul(hsum[:], hsum[:], 1.0 / T)
            nc.sync.dma_start(out_d[:], hsum[:])

    _nc_cache[0] = nc
    return nc


# ---------------------------------------------------------------------------
# host-side helpers (exact fp32)
# ---------------------------------------------------------------------------


def _conv_np(x, w, pad):
    """x [Ci,H,W], w [Co,Ci,kh,kw] -> [Co,Ho,Wo] fp32, matmul per offset."""
    Co, Ci, kh, kw = w.shape
    Hh, Ww = x.shape[1], x.shape[2]
    xp = np.zeros((Ci, Hh + 2 * pad, Ww + 2 * pad), np.float32)
    xp[:, pad : pad + Hh, pad : pad + Ww] = x
    Ho = Hh + 2 * pad - kh + 1
    Wo = Ww + 2 * pad - kw + 1
    out = np.zeros((Co, Ho * Wo), np.float32)
    for dy in range(kh):
        for dx in range(kw):
            patch = xp[:, dy : dy + Ho, dx : dx + Wo].reshape(Ci, -1)
            out += w[:, :, dy, dx] @ patch
    return out.reshape(Co, Ho, Wo)


def kernel(
    rgb_a,
    confidence_a,
    phi_x_w,
    phi_h_w,
    lstm_w,
    lstm_b,
    conv1_w,
    conv1_b,
    conv2_w,
    conv2_b,
):
    rgb_a = np.asarray(rgb_a, np.float32)
    confidence_a = np.asarray(confidence_a, np.float32)
    lstm_w = np.asarray(lstm_w, np.float32)
    lstm_b = np.asarray(lstm_b, np.float32)

    # --- attention prep (att_h is a constant shift inside softmax -> drop it)
    s = rgb_a * confidence_a
    s = (s - s.min()) / (s.max() - s.min())
    att_x = s.mean(axis=(2, 3)) @ np.asarray(phi_x_w, np.float32)[0]
    e = np.exp(att_x - att_x.max())
    wts = e / e.sum()
    x_t = (s * wts[:, None, None, None]).sum(0) / T  # [3,H,W]

    # --- x-path conv (one-time) and weight layout for the device
    wx = lstm_w[:, :3]
    whh = lstm_w[:, 3:]  # [512,128,5,5]
    gx_full = _conv_np(x_t, wx, 2) + lstm_b[:, None, None]  # [512,64,64]
    gx_in = np.ascontiguousarray(
        gx_full.reshape(4, 128, HW), dtype=np.float32
    )
    # wh[i, og*25*128 + off*128 + o] = whh[og*128+o, i, ky, kx]
    wh_in = np.ascontiguousarray(
        whh.reshape(4, 128, 128, 5, 5).transpose(2, 0, 3, 4, 1).reshape(128, -1)
    ).astype(ml_dtypes.bfloat16)

    nc = build_nc()
    in_map = {"wh": wh_in, "gx": gx_in}
    res = run_bass_kernel_spmd(
        nc,
        [dict(in_map) for _ in range(N_CORES)],
        core_ids=list(range(N_CORES)),
    )
    hmean = res.results[0]["hmean"].reshape(HS, H, W).astype(np.float32)

    # --- CNN tail (host, exact fp32)
    hp = np.full((HS, H + 1, W + 1), -np.inf, np.float32)
    hp[:, :H, :W] = hmean
    views = [
        hp[:, dy : dy + 63 + 1 : 2, dx : dx + 63 + 1 : 2]
        for dy in range(3)
        for dx in range(3)
    ]
    p = np.max(np.stack([v[:, :32, :32] for v in views]), axis=0)

    def sig(v):
        return 1.0 / (1.0 + np.exp(-v))

    y = sig(
        _conv_np(p, np.asarray(conv1_w, np.float32), 3)
        + np.asarray(conv1_b, np.float32)[:, None, None]
    )
    y = sig(
        _conv_np(y, np.asarray(conv2_w, np.float32), 0)
        + np.asarray(conv2_b, np.float32)[:, None, None]
    )
    v = y.sum(axis=(1, 2))
    pred = v / max(np.linalg.norm(v), 1e-12)
    return pred[None].astype(np.float32)



# revision 7
# speedup vs baseline: 3.4447x; 3.4447x over previous
"""AttentionTCCNet Trainium2 Bass kernel — spatially sharded over 8 cores.

Key algebraic fact: the per-step attention adds a *scalar* (att_h) to every
softmax logit, so the softmax weights — and hence the attended frame x_t —
are constant across the 16 recurrence steps.  The computation reduces to a
ConvLSTM recurrence whose per-step cost is a 128->512ch 5x5 conv over the
hidden state, plus a one-time x-path conv and a small CNN tail.

Sharding: each of the 8 cores owns an 8-row strip of the 64x64 image.  The
5x5 conv needs a 2-row halo from each neighbour every step.  Halo exchange:
each core writes its boundary rows into per-destination slots of an internal
DRAM buffer using dynamic-offset DMA (slot index = partition_id +/- 1, edge
cores skip via OOB bounds check), then one 8-way ReduceScatter(add) delivers
exactly the 4 halo rows each core needs (all other contributions are zero).

Boundary-row matmuls are split pre-halo/post-halo so the collective latency
overlaps interior compute: only the ky rows touching halo data wait for the
exchange.

Host: input attention prep, the tiny 3-channel x-path conv, and the CNN tail
(maxpool + 2 convs + normalize), all exact fp32.
"""

import numpy as np
import ml_dtypes

import concourse.bass as bass
import concourse.mybir as mybir
import concourse.tile as tile
from concourse.bass_utils import run_bass_kernel_spmd

# ---------------------------------------------------------------------------
# Workaround for this container's walrus accepting only ONE SyncWait per
# instruction: split any multi-wait instruction emitted by Tile's semaphore
# assigner into single-wait NoOp carriers inserted immediately before it.
# ---------------------------------------------------------------------------
from concourse.tile import ScopedClock

_MAX_WAITS = 1
_wsplit_counter = [0]


def _split_waits_in_list(insts):
    new = []
    for inst in insts:
        si = getattr(inst, "sync_info", None)
        if si is not None and si.on_wait and len(si.on_wait) > _MAX_WAITS:
            waits = list(si.on_wait)
            for w in waits[:-_MAX_WAITS]:
                _wsplit_counter[0] += 1
                new.append(
                    mybir.InstNoOp(
                        name=f"I-wsplit-{_wsplit_counter[0]}",
                        engine=inst.engine,
                        sync_info=mybir.SyncInfo(on_wait=[w], on_update=[]),
                    )
                )
            si.on_wait = waits[-_MAX_WAITS:]
        new.append(inst)
    insts[:] = new


_orig_lower = tile.TileContext._lower_ordered_insts


def _patched_lower(self, ordered):
    for insts in ordered.values():
        _split_waits_in_list(insts)
    return _orig_lower(self, ordered)


def _patched_drain_and_barrier(self, tick_clock, wait_clock):
    nc = self.nc
    drain_inst = nc.sync.drain()
    wait_clock.add_sem_waits(
        drain_inst.ins, ScopedClock({None: tick_clock.global_clock})
    )
    si = drain_inst.ins.sync_info
    if si is not None and si.on_wait and len(si.on_wait) > _MAX_WAITS:
        waits = list(si.on_wait)
        si.on_wait = waits[:_MAX_WAITS]
        for w in waits[_MAX_WAITS:]:
            extra = nc.sync.drain()
            extra.ins.sync_info = mybir.SyncInfo(on_wait=[w], on_update=[])
    nc.all_engine_barrier()
    assert self.sems is not None
    popped = nc._tile_sem_poison_stack.pop()
    assert popped is self._sem_poison
    nc.clear_and_free_semaphores(list(self.sems.allocated().values()))
    nc.all_engine_barrier()


if tile.TileContext._lower_ordered_insts is not _patched_lower:
    tile.TileContext._lower_ordered_insts = _patched_lower
    tile.TileContext._drain_and_barrier = _patched_drain_and_barrier

# ---------------------------------------------------------------------------

N_CORES = 8
T, HS, H, W = 16, 128, 64, 64
HW = H * W
ROWS = 8            # rows per core
CF = ROWS * W       # free size per core = 512
PADW = 68           # 64 + 2*2 padded row layout
PADR = 12           # 2 halo + 8 own + 2 halo rows in h_pad

FP32 = mybir.dt.float32
BF16 = mybir.dt.bfloat16

SIG = mybir.ActivationFunctionType.Sigmoid
TANH = mybir.ActivationFunctionType.Tanh

_nc_cache = [None]


def build_nc():
    if _nc_cache[0] is not None:
        return _nc_cache[0]
    nc = bass.Bass(num_devices=N_CORES)
    wh_d = nc.dram_tensor("wh", [128, 4 * 25 * 128], BF16, kind="ExternalInput")
    gx_d = nc.dram_tensor("gx", [4, 128, CF], FP32, kind="ExternalInput")
    h1h_d = nc.dram_tensor("h1halo", [128, 4, 64], BF16, kind="ExternalInput")
    out_d = nc.dram_tensor("hmean", [128, CF], FP32, kind="ExternalOutput")

    with tile.TileContext(nc) as tc:
        with (
            tc.tile_pool(name="const", bufs=1) as cpool,
            tc.tile_pool(name="tmp", bufs=2) as tpool,
            tc.tile_pool(name="psum", bufs=2, space="PSUM") as ppool,
            tc.tile_pool(name="dram", bufs=1, space="DRAM") as dpool,
        ):
            wh = cpool.tile([128, 4 * 25 * 128], BF16)
            gx = cpool.tile([128, 4, CF], FP32)
            c_st = cpool.tile([128, CF], FP32)
            hsum = cpool.tile([128, CF], FP32)
            hbuf0 = cpool.tile([128, PADR, PADW], BF16, tag="hpad0")
            hbuf1 = cpool.tile([128, PADR, PADW], BF16, tag="hpad1")
            hbufs = [hbuf0, hbuf1]
            zeros = cpool.tile([128, 2048], BF16)

            xin = dpool.tile([8, 128, 256], BF16)
            xout = dpool.tile([128, 256], BF16)

            nc.sync.dma_start(gx[:], gx_d.ap().rearrange("a p h -> p a h"))
            nc.sync.dma_start(wh[:], wh_d[:])
            nc.gpsimd.memset(zeros[:], 0.0)
            for hb in hbufs:
                nc.gpsimd.memset(hb[:], 0.0)
            nc.sync.dma_start(xin[:], zeros[:])

            # slot indices for halo sends (pid +/- 1); OOB at the edges skips
            pid_s = nc.sync.partition_id()
            rm1 = nc.sync.alloc_register("rm1")
            nc.sync.reg_add(rm1, pid_s, -1)
            off_m1 = nc.snap(rm1, engines=None)
            pid_a = nc.scalar.partition_id()
            rp1 = nc.scalar.alloc_register("rp1")
            nc.scalar.reg_add(rp1, pid_a, 1)
            off_p1 = nc.snap(rp1, engines=None)

            # out-row blocks: (name, r0, nrows, pre_kys, post_kys)
            # moving rows for out rows r0..r0+nr-1 at ky = hpad rows r0+ky .. r0+ky+nr-1
            BLK_TOP = ("bt", 0, 2, (2, 3, 4), (0, 1))
            BLK_BOT = ("bb", 6, 2, (0, 1, 2), (3, 4))
            BLK_MID = ("bm", 2, 4, (0, 1, 2, 3, 4), ())

            def mm(ps, pcol0, og, h_in, r0, nr, ky, kx, start, stop):
                off = ky * 5 + kx
                base = (og * 25 + off) * 128
                nc.tensor.matmul(
                    ps[:, pcol0 : pcol0 + nr * 64],
                    wh[:, base : base + 128],
                    h_in[:, r0 + ky : r0 + ky + nr, kx : kx + 64],
                    start=start,
                    stop=stop,
                )

            # --- the recurrence ---------------------------------------------
            ps_pre = {}  # block name -> psum tile with pre-halo partials

            def emit_pre(name, r0, nr, kys, h_in):
                """allocate psum for block and run pre-halo matmuls"""
                ncols = nr * 64
                ps = ppool.tile([128, 4, ncols], FP32, tag=f"ps_{name}")
                for og in range(4):
                    first = True
                    for ky in kys:
                        for kx in range(5):
                            nc.tensor.matmul(
                                ps[:, og, :],
                                wh[:, ((og * 25 + ky * 5 + kx) * 128) :][
                                    :, 0:128
                                ],
                                h_in[:, r0 + ky : r0 + ky + nr, kx : kx + 64],
                                start=first,
                                stop=False,
                            )
                            first = False
                ps_pre[name] = ps
                return ps

            def emit_post(name, r0, nr, kys, h_in):
                ps = ps_pre[name]
                for og in range(4):
                    n = len(kys) * 5
                    i = 0
                    for ky in kys:
                        for kx in range(5):
                            i += 1
                            nc.tensor.matmul(
                                ps[:, og, :],
                                wh[:, ((og * 25 + ky * 5 + kx) * 128) :][
                                    :, 0:128
                                ],
                                h_in[:, r0 + ky : r0 + ky + nr, kx : kx + 64],
                                start=False,
                                stop=(i == n),
                            )
                return ps

            def pointwise(t, ps, col0, ncols, h_out, hr0):
                """gates -> activations -> LSTM update for columns
                [col0, col0+ncols) (ps None => gates == gx, i.e. t == 0)."""
                acts = []
                for og in range(4):
                    fn = TANH if og == 3 else SIG
                    a = tpool.tile([128, ncols], FP32, tag=f"act{og}_{ncols}")
                    if ps is None:
                        nc.scalar.activation(
                            a[:], gx[:, og, col0 : col0 + ncols], fn
                        )
                    else:
                        g_sb = tpool.tile(
                            [128, ncols], FP32, tag=f"gsb{og}_{ncols}"
                        )
                        nc.vector.tensor_add(
                            g_sb[:], ps[:, og, :], gx[:, og, col0 : col0 + ncols]
                        )
                        nc.scalar.activation(a[:], g_sb[:], fn)
                    acts.append(a)
                i_s, f_s, o_s, g_t = acts
                c_sl = c_st[:, col0 : col0 + ncols]
                m2 = tpool.tile([128, ncols], FP32, tag=f"m2_{ncols}")
                nc.vector.tensor_mul(m2[:], i_s[:], g_t[:])
                if t == 0:
                    nc.vector.tensor_copy(c_sl, m2[:])
                else:
                    m1 = tpool.tile([128, ncols], FP32, tag=f"m1_{ncols}")
                    nc.vector.tensor_mul(m1[:], f_s[:], c_sl)
                    nc.vector.tensor_add(c_sl, m1[:], m2[:])
                tc_t = tpool.tile([128, ncols], FP32, tag=f"tc_{ncols}")
                nc.scalar.activation(tc_t[:], c_sl, TANH)
                hf = tpool.tile([128, ncols], FP32, tag=f"hf_{ncols}")
                nc.vector.tensor_mul(hf[:], o_s[:], tc_t[:])
                hs_sl = hsum[:, col0 : col0 + ncols]
                if t == 0:
                    nc.vector.tensor_copy(hs_sl, hf[:])
                else:
                    nc.vector.tensor_add(hs_sl, hs_sl, hf[:])
                if h_out is not None:
                    nr = ncols // 64
                    nc.vector.tensor_copy(
                        h_out[:, hr0 : hr0 + nr, 2:66],
                        hf[:].rearrange("p (r c) -> p r c", r=nr),
                    )

            def send_and_rs(h_cur):
                # my top rows -> slot (pid-1) bottom-area; bottom rows -> slot
                # (pid+1) top-area.  OOB (edge cores) skipped.
                nc.sync.dma_start(
                    xin[bass.ds(off_m1, 1), :, 128:256],
                    h_cur[:, 2:4, 2:66],
                    bounds_check="skip_entire_dma",
                )
                nc.scalar.dma_start(
                    xin[bass.ds(off_p1, 1), :, 0:128],
                    h_cur[:, 8:10, 2:66],
                    bounds_check="skip_entire_dma",
                )
                nc.gpsimd.collective_compute(
                    "ReduceScatter",
                    mybir.AluOpType.add,
                    replica_groups=[list(range(N_CORES))],
                    ins=[xin[:].opt()],
                    outs=[xout[:].opt()],
                )
                # halo arrives: top halo rows 0:2, bottom halo rows 10:12
                nc.sync.dma_start(h_cur[:, 0:2, 2:66], xout[:, 0:128])
                nc.sync.dma_start(h_cur[:, 10:12, 2:66], xout[:, 128:256])

            # ---- t = 0: h == 0, gates == gx --------------------------------
            h_cur = hbufs[0]
            nc.sync.dma_start(h_cur[:, 0:2, 2:66], h1h_d[:, 0:2, :])
            nc.sync.dma_start(h_cur[:, 10:12, 2:66], h1h_d[:, 2:4, :])
            pointwise(0, None, 0, 128, h_cur, 2)       # boundary top rows 0,1
            pointwise(0, None, 384, 128, h_cur, 8)     # boundary bottom rows 6,7
            pointwise(0, None, 128, 256, h_cur, 4)     # middle rows 2..5
            nm, r0, nr, pre_k, post_k = BLK_TOP
            emit_pre(nm, r0, nr, pre_k, h_cur)
            nm, r0, nr, pre_k, post_k = BLK_BOT
            emit_pre(nm, r0, nr, pre_k, h_cur)

            # ---- t = 1..15 --------------------------------------------------
            for t in range(1, T):
                h_prev = hbufs[(t - 1) % 2]
                h_cur = hbufs[t % 2] if t < T - 1 else None

                # post-halo boundary matmuls + boundary pointwise
                for (nm, r0, nr, pre_k, post_k), col0, hr0 in (
                    (BLK_TOP, 0, 2),
                    (BLK_BOT, 384, 8),
                ):
                    ps = emit_post(nm, r0, nr, post_k, h_prev)
                    pointwise(t, ps, col0, nr * 64, h_cur, hr0)

                if 1 <= t < T - 1:
                    send_and_rs(h_cur)

                # middle matmuls + pointwise (independent of the collective)
                nm, r0, nr, kys, _ = BLK_MID
                ncols = nr * 64
                ps = ppool.tile([128, 4, ncols], FP32, tag=f"ps_{nm}")
                for og in range(4):
                    i = 0
                    for ky in kys:
                        for kx in range(5):
                            i += 1
                            nc.tensor.matmul(
                                ps[:, og, :],
                                wh[:, ((og * 25 + ky * 5 + kx) * 128) :][
                                    :, 0:128
                                ],
                                h_prev[:, r0 + ky : r0 + ky + nr, kx : kx + 64],
                                start=(i == 1),
                                stop=(i == 25),
                            )
                pointwise(t, ps, 128, ncols, h_cur, 4)

                # pre-halo boundary matmuls for step t+1
                if t < T - 1:
                    nm, r0, nr, pre_k, post_k = BLK_TOP
                    emit_pre(nm, r0, nr, pre_k, h_cur)
                    nm, r0, nr, pre_k, post_k = BLK_BOT
                    emit_pre(nm, r0, nr, pre_k, h_cur)

            nc.scalar.mul(hsum[:], hsum[:], 1.0 / T)
            nc.sync.dma_start(out_d[:], hsum[:])

    _nc_cache[0] = nc
    return nc


# ---------------------------------------------------------------------------
# host-side helpers (exact fp32)
# ---------------------------------------------------------------------------


def _conv_np(x, w, pad):
    """x [Ci,H,W], w [Co,Ci,kh,kw] -> [Co,Ho,Wo] fp32, matmul per offset."""
    Co, Ci, kh, kw = w.shape
    Hh, Ww = x.shape[1], x.shape[2]
    xp = np.zeros((Ci, Hh + 2 * pad, Ww + 2 * pad), np.float32)
    xp[:, pad : pad + Hh, pad : pad + Ww] = x
    Ho = Hh + 2 * pad - kh + 1
    Wo = Ww + 2 * pad - kw + 1
    out = np.zeros((Co, Ho * Wo), np.float32)
    for dy in range(kh):
        for dx in range(kw):
            patch = xp[:, dy : dy + Ho, dx : dx + Wo].reshape(Ci, -1)
            out += w[:, :, dy, dx] @ patch
    return out.reshape(Co, Ho, Wo)


def kernel(
    rgb_a,
    confidence_a,
    phi_x_w,
    phi_h_w,
    lstm_w,
    lstm_b,
    conv1_w,
    conv1_b,
    conv2_w,
    conv2_b,
):
    rgb_a = np.asarray(rgb_a, np.float32)
    confidence_a = np.asarray(confidence_a, np.float32)
    lstm_w = np.asarray(lstm_w, np.float32)
    lstm_b = np.asarray(lstm_b, np.float32)

    # --- attention prep (att_h is a constant shift inside softmax -> drop it)
    s = rgb_a * confidence_a
    s = (s - s.min()) / (s.max() - s.min())
    att_x = s.mean(axis=(2, 3)) @ np.asarray(phi_x_w, np.float32)[0]
    e = np.exp(att_x - att_x.max())
    wts = e / e.sum()
    x_t = (s * wts[:, None, None, None]).sum(0) / T  # [3,H,W]

    # --- x-path conv (one-time) and weight layout for the device
    wx = lstm_w[:, :3]
    whh = lstm_w[:, 3:]  # [512,128,5,5]
    gx_full = _conv_np(x_t, wx, 2) + lstm_b[:, None, None]  # [512,64,64]
    gx_rows = gx_full.reshape(4, 128, H, W)
    # wh[i, og*25*128 + off*128 + o] = whh[og*128+o, i, ky, kx]
    wh_in = np.ascontiguousarray(
        whh.reshape(4, 128, 128, 5, 5).transpose(2, 0, 3, 4, 1).reshape(128, -1)
    ).astype(ml_dtypes.bfloat16)

    # h1 = sigmoid(o)*tanh(sigmoid(i)*tanh(g)) with gates == gx (h0 == 0)
    def _sig(v):
        return 1.0 / (1.0 + np.exp(-v))

    gxr = gx_rows  # [4, 128, 64, 64] (i, f, o, g)
    h1 = _sig(gxr[2]) * np.tanh(_sig(gxr[0]) * np.tanh(gxr[3]))  # [128,64,64]
    h1 = h1.astype(ml_dtypes.bfloat16)

    nc = build_nc()
    in_maps = []
    for k in range(N_CORES):
        gx_k = np.ascontiguousarray(
            gx_rows[:, :, k * ROWS : (k + 1) * ROWS, :].reshape(4, 128, CF),
            dtype=np.float32,
        )
        h1h = np.zeros((128, 4, 64), dtype=ml_dtypes.bfloat16)
        if k > 0:
            h1h[:, 0:2, :] = h1[:, k * ROWS - 2 : k * ROWS, :]
        if k < N_CORES - 1:
            h1h[:, 2:4, :] = h1[:, (k + 1) * ROWS : (k + 1) * ROWS + 2, :]
        in_maps.append({"wh": wh_in, "gx": gx_k, "h1halo": h1h})
    res = run_bass_kernel_spmd(nc, in_maps, core_ids=list(range(N_CORES)))
    hmean = np.concatenate(
        [
            res.results[k]["hmean"].reshape(HS, ROWS, W).astype(np.float32)
            for k in range(N_CORES)
        ],
        axis=1,
    )  # [128, 64, 64]

    # --- CNN tail (host, exact fp32)
    hp = np.full((HS, H + 1, W + 1), -np.inf, np.float32)
    hp[:, :H, :W] = hmean
    views = [
        hp[:, dy : dy + 63 + 1 : 2, dx : dx + 63 + 1 : 2]
        for dy in range(3)
        for dx in range(3)
    ]
    p = np.max(np.stack([v[:, :32, :32] for v in views]), axis=0)

    def sig(v):
        return 1.0 / (1.0 + np.exp(-v))

    y = sig(
        _conv_np(p, np.asarray(conv1_w, np.float32), 3)
        + np.asarray(conv1_b, np.float32)[:, None, None]
    )
    y = sig(
        _conv_np(y, np.asarray(conv2_w, np.float32), 0)
        + np.asarray(conv2_b, np.float32)[:, None, None]
    )
    v = y.sum(axis=(1, 2))
    pred = v / max(np.linalg.norm(v), 1e-12)
    return pred[None].astype(np.float32)
